# revision 1
# baseline (speedup 1.0000x reference)
"""HET RelationalAttLayer on 8 trn2 NeuronCores.

Strategy (relation-per-core, all per-core variation is data-driven):
  core r handles relation r (102400 edges each).
  Phase 1: Zext = X @ [W_r | W_r@attn_l_r | W_r@attn_r_r]  -> [50176, 136] bf16
  Phase 2: per dst-sorted 128-edge tile: indirect-gather Zext rows by src,
           gather er by dst, q = exp(leaky_relu(el+er)), rhs = [q*Z | q],
           one-hot (dst_local) matmul -> per-128-node-block partial sums,
           scatter-add (CCE) into S [50176, 132] f32 in HBM.
  Phase 3: ReduceScatter(add) S across 8 cores, normalize by denom, + bias.
Host does index preprocessing only (sort edges by dst, bake gather/scatter
index tables and tile structure); all float math runs on device.
"""
import numpy as np

N_NODES = 50000
NPAD = 50176          # 392 blocks of 128
NBLK = NPAD // 128    # 392
N_EDGES = 819200
NUM_RELS = 8
N_HEADS = 4
IN_FEAT = 256
OUT_FEAT = 128
HEAD_DIM = 32
LEAKY = 0.2
NCORES = 8
EPR = N_EDGES // NUM_RELS
ZC = 136              # zext row: 128 Z | 4 el | 4 er
SC = 132              # S row: 128 feat | 4 denom
TPB = 3               # tiles per 128-node block (fixed, program-static)
BPC = 28              # blocks per index chunk
TC = TPB * BPC        # tiles per index chunk (84)
SHARD = NPAD // NCORES  # 6272

_CACHE = {}


# ----------------------------------------------------------------- host prep
def _edge_plan(src, dst):
    """dst-sorted edges packed into TPB fixed 128-edge tiles per 128-node
    block; pad slots have src=0, dstf=-1 (one-hot row all zero)."""
    order = np.argsort(dst, kind='stable')
    src, dst = src[order], dst[order]
    lo = np.searchsorted(dst, np.arange(NBLK) * 128)
    hi = np.searchsorted(dst, np.minimum(np.arange(1, NBLK + 1) * 128, NPAD))
    cnt = hi - lo
    assert cnt.max() <= TPB * 128, f"block overflow: {cnt.max()}"
    T = NBLK * TPB
    t_src = np.zeros((T, 128), np.int32)
    t_dstf = np.full((T, 128), -1.0, np.float32)
    for b in range(NBLK):
        e0, e1 = int(lo[b]), int(hi[b])
        for k in range(TPB):
            a = e0 + k * 128
            bnd = min(a + 128, e1)
            if bnd <= a:
                break
            t = b * TPB + k
            t_src[t, :bnd - a] = src[a:bnd]
            t_dstf[t, :bnd - a] = (dst[a:bnd] - b * 128).astype(np.float32)
    return t_src, t_dstf


def _prep(inputs, conv_weights, attn_l, attn_r, h_bias, row_idx, col_idx):
    X = np.asarray(inputs, np.float32)
    W = np.asarray(conv_weights, np.float32)
    al = np.asarray(attn_l, np.float32)
    ar = np.asarray(attn_r, np.float32)
    row = np.asarray(row_idx).astype(np.int64)
    col = np.asarray(col_idx).astype(np.int64)

    # X^T tiles [NBLK, 2, 128, 128]: [b,k,i,j] = X[b*128+j, k*128+i]
    Xp = np.zeros((NPAD, IN_FEAT), np.float32)
    Xp[:N_NODES] = X
    xt = Xp.reshape(NBLK, 128, 2, 128).transpose(0, 2, 3, 1).copy()

    # per-relation plans (T = NBLK*TPB, static)
    T = NBLK * TPB
    nch = T // TC
    blk_of_tile = np.repeat(np.arange(NBLK, dtype=np.int32), TPB)
    per_core = []
    for r in range(NUM_RELS):
        s = row[r * EPR:(r + 1) * EPR].astype(np.int32)
        d = col[r * EPR:(r + 1) * EPR].astype(np.int32)
        src_all, dstf_all = _edge_plan(s, d)
        gdst = np.where(dstf_all >= 0,
                        blk_of_tile[:, None] * 128 + dstf_all.astype(np.int32),
                        0).astype(np.int32)

        def cm(a, dt):
            return np.ascontiguousarray(
                a.reshape(nch, TC, 128).transpose(0, 2, 1)).astype(dt)
        Wf = np.concatenate([W[r, h] for h in range(N_HEADS)], axis=1)
        wl = np.stack([W[r, h] @ al[r, h] for h in range(N_HEADS)], axis=1)
        wr = np.stack([W[r, h] @ ar[r, h] for h in range(N_HEADS)], axis=1)
        wext = np.concatenate([Wf, wl, wr], axis=1).astype(np.float32)
        per_core.append(dict(
            wext=wext,
            gsrc=cm(src_all, np.int32),
            gdst=cm(gdst, np.int32),
            dstf=cm(dstf_all, np.float32),
        ))
    consts = dict(
        iota=np.broadcast_to(np.arange(128, dtype=np.float32), (128, 128)).copy(),
        bias=np.broadcast_to(np.asarray(h_bias, np.float32), (128, 128)).copy(),
    )
    return xt, per_core, consts, T


# ------------------------------------------------------------- device program
def _build(T):
    import concourse.bacc as bacc
    import concourse.tile as tile
    from concourse import mybir
    from concourse.bass import IndirectOffsetOnAxis
    from concourse.bass_types import AP

    f32 = mybir.dt.float32
    bf16 = mybir.dt.bfloat16
    i32 = mybir.dt.int32

    nc = bacc.Bacc("TRN2", target_bir_lowering=False, debug=False,
                   num_devices=NCORES)
    nch = T // TC

    xt = nc.dram_tensor("xt", [NBLK, 2, 128, 128], f32, kind="ExternalInput")
    wext = nc.dram_tensor("wext", [IN_FEAT, ZC], f32, kind="ExternalInput")
    gsrc = nc.dram_tensor("gsrc", [nch, 128, TC], i32, kind="ExternalInput")
    gdst = nc.dram_tensor("gdst", [nch, 128, TC], i32, kind="ExternalInput")
    dstf = nc.dram_tensor("dstf", [nch, 128, TC], f32, kind="ExternalInput")
    iota = nc.dram_tensor("iota", [128, 128], bf16, kind="ExternalInput")
    bias = nc.dram_tensor("bias", [128, 128], f32, kind="ExternalInput")
    out = nc.dram_tensor("out", [SHARD, OUT_FEAT], f32, kind="ExternalOutput")

    with tile.TileContext(nc) as tc:
        with tc.tile_pool(name="dram", bufs=1, space="DRAM") as dram, \
             tc.tile_pool(name="consts", bufs=1) as cpool, \
             tc.tile_pool(name="ph1", bufs=4) as ph1, \
             tc.tile_pool(name="ph1p", bufs=4, space="PSUM") as ph1p, \
             tc.tile_pool(name="idx", bufs=3) as idxp, \
             tc.tile_pool(name="gat", bufs=4) as gat, \
             tc.tile_pool(name="edg", bufs=4) as edg, \
             tc.tile_pool(name="ohp", bufs=6) as ohp, \
             tc.tile_pool(name="ph2p", bufs=4, space="PSUM") as ph2p, \
             tc.tile_pool(name="outp", bufs=6) as outp:

            zext = dram.tile([NPAD, ZC], bf16)
            ern = dram.tile([NPAD, 4], bf16)
            s_acc = dram.tile([NPAD, SC], f32)
            s_red = dram.tile([SHARD, SC], f32)
            assert zext[:].offset == 0 or True

            # consts
            w_sb = cpool.tile([128, 2 * ZC], f32, tag="wext")
            wap = wext[:]
            nc.sync.dma_start(
                w_sb[:], AP(wap.tensor, 0, [[ZC, 128], [128 * ZC, 2], [1, ZC]]))
            iota_sb = cpool.tile([128, 128], bf16, tag="iota")
            nc.sync.dma_start(iota_sb[:], iota[:])
            bias_sb = cpool.tile([128, 128], f32, tag="bias")
            nc.sync.dma_start(bias_sb[:], bias[:])

            # ---------------- Phase 1: Zext GEMM ----------------
            for b in range(NBLK):
                lh0 = ph1.tile([128, 128], f32, tag="lh")
                lh1 = ph1.tile([128, 128], f32, tag="lh")
                nc.sync.dma_start(lh0[:], xt[b, 0])
                nc.sync.dma_start(lh1[:], xt[b, 1])
                ps = ph1p.tile([128, ZC], f32, space="PSUM", tag="zps")
                nc.tensor.matmul(out=ps[:], lhsT=lh0[:], rhs=w_sb[:, 0:ZC],
                                 start=True, stop=False)
                nc.tensor.matmul(out=ps[:], lhsT=lh1[:], rhs=w_sb[:, ZC:2 * ZC],
                                 start=False, stop=True)
                zr = ph1.tile([128, ZC], bf16, tag="zr")
                nc.scalar.copy(zr[:], ps[:])
                nc.sync.dma_start(zext[:][b * 128:(b + 1) * 128, :], zr[:])
                nc.sync.dma_start(ern[:][b * 128:(b + 1) * 128, :],
                                  zr[:, 128 + 4:ZC])

            # ---------------- Phase 2: edges ----------------
            for c in range(nch):
                gs = idxp.tile([128, TC], i32, tag="gs")
                gd = idxp.tile([128, TC], i32, tag="gd")
                df = idxp.tile([128, TC], f32, tag="df")
                nc.sync.dma_start(gs[:], gsrc[c])
                nc.sync.dma_start(gd[:], gdst[c])
                nc.sync.dma_start(df[:], dstf[c])
                for bb in range(BPC):
                    b = c * BPC + bb
                    t0 = bb * TPB
                    G = gat.tile([128, TPB * ZC], bf16, tag="G")
                    erg = edg.tile([128, TPB * 4], bf16, tag="erg")
                    for k in range(TPB):
                        nc.gpsimd.indirect_dma_start(
                            out=G[:, k * ZC:(k + 1) * ZC], out_offset=None,
                            in_=zext[:],
                            in_offset=IndirectOffsetOnAxis(
                                ap=gs[:, t0 + k:t0 + k + 1], axis=0))
                        nc.gpsimd.indirect_dma_start(
                            out=erg[:, k * 4:(k + 1) * 4], out_offset=None,
                            in_=ern[:],
                            in_offset=IndirectOffsetOnAxis(
                                ap=gd[:, t0 + k:t0 + k + 1], axis=0))
                    gap = G[:]
                    el_view = AP(gap.tensor, gap.offset + 128,
                                 [gap.ap[0], [ZC, TPB], [1, 4]])
                    q0 = edg.tile([128, TPB * 4], bf16, tag="q0")
                    nc.vector.tensor_tensor(q0[:], el_view, erg[:],
                                            op=mybir.AluOpType.add)
                    qs = edg.tile([128, TPB * 4], bf16, tag="qs")
                    nc.vector.tensor_scalar(qs[:], q0[:], LEAKY, None,
                                            op0=mybir.AluOpType.mult)
                    q1 = edg.tile([128, TPB * 4], bf16, tag="q1")
                    nc.vector.tensor_tensor(q1[:], q0[:], qs[:],
                                            op=mybir.AluOpType.max)
                    q = edg.tile([128, TPB * 4], bf16, tag="q")
                    nc.scalar.activation(q[:], q1[:],
                                         mybir.ActivationFunctionType.Exp)
                    rhs = gat.tile([128, TPB * SC], bf16, tag="rhs")
                    rap = rhs[:]
                    z_in = AP(gap.tensor, gap.offset,
                              [gap.ap[0], [ZC, TPB], [1, 128]])
                    qap = q[:]
                    q_rep = AP(qap.tensor, qap.offset,
                               [qap.ap[0], [4, TPB], [1, 4], [0, HEAD_DIM]])
                    z_out = AP(rap.tensor, rap.offset,
                               [rap.ap[0], [SC, TPB], [1, 128]])
                    nc.vector.tensor_tensor(z_out, z_in, q_rep,
                                            op=mybir.AluOpType.mult)
                    q_out = AP(rap.tensor, rap.offset + 128,
                               [rap.ap[0], [SC, TPB], [1, 4]])
                    nc.vector.tensor_copy(q_out, qap)
                    ps2 = ph2p.tile([128, SC], f32, space="PSUM", tag="sps")
                    for k in range(TPB):
                        oh = ohp.tile([128, 128], bf16, tag="oh")
                        nc.vector.tensor_scalar(
                            oh[:], iota_sb[:], df[:, t0 + k:t0 + k + 1], None,
                            op0=mybir.AluOpType.is_equal)
                        nc.tensor.matmul(
                            out=ps2[:], lhsT=oh[:],
                            rhs=AP(rap.tensor, rap.offset + k * SC,
                                   [rap.ap[0], [1, SC]]),
                            start=(k == 0), stop=(k == TPB - 1))
                    so = outp.tile([128, SC], f32, tag="so")
                    nc.scalar.copy(so[:], ps2[:])
                    nc.sync.dma_start(
                        s_acc[:][b * 128:(b + 1) * 128, :], so[:])

            # ---------------- Phase 3: reduce + normalize ----------------
            nc.gpsimd.collective_compute(
                "ReduceScatter", mybir.AluOpType.add,
                replica_groups=[list(range(NCORES))],
                ins=[s_acc[:].opt()], outs=[s_red[:].opt()])
            for i in range(SHARD // 128):
                st = outp.tile([128, SC], f32, tag="st")
                nc.sync.dma_start(st[:], s_red[:][i * 128:(i + 1) * 128, :])
                dg = outp.tile([128, 4], f32, tag="dg")
                nc.vector.tensor_scalar(dg[:], st[:, 128:SC], 1e-30, None,
                                        op0=mybir.AluOpType.max)
                rc = outp.tile([128, 4], f32, tag="rc")
                nc.vector.reciprocal(rc[:], dg[:])
                ot = outp.tile([128, OUT_FEAT], f32, tag="ot")
                rcap = rc[:]
                rc_rep = AP(rcap.tensor, rcap.offset,
                            [rcap.ap[0], [1, 4], [0, HEAD_DIM]])
                nc.vector.tensor_tensor(ot[:], st[:, 0:128], rc_rep,
                                        op=mybir.AluOpType.mult)
                ot2 = outp.tile([128, OUT_FEAT], f32, tag="ot2")
                nc.vector.tensor_tensor(ot2[:], ot[:], bias_sb[:],
                                        op=mybir.AluOpType.add)
                nc.sync.dma_start(out[:][i * 128:(i + 1) * 128, :], ot2[:])

    nc.compile()
    return nc


# ------------------------------------------------------------------- entry
def kernel(inputs, conv_weights, attn_l, attn_r, h_bias, row_idx, col_idx,
           _trace=False, _tmpdir=None):
    import ml_dtypes
    xt, per_core, consts, T = _prep(inputs, conv_weights, attn_l, attn_r,
                                    h_bias, row_idx, col_idx)
    if ('nc', T) not in _CACHE:
        _CACHE[('nc', T)] = _build(T)
    nc = _CACHE[('nc', T)]

    bf = ml_dtypes.bfloat16
    in_maps = []
    for r in range(NCORES):
        pc = per_core[r]
        in_maps.append(dict(
            xt=xt, wext=pc['wext'],
            gsrc=pc['gsrc'], gdst=pc['gdst'],
            dstf=pc['dstf'],
            iota=consts['iota'].astype(bf), bias=consts['bias'],
        ))

    from concourse import bass_utils
    res = bass_utils.run_bass_kernel_spmd(
        nc, in_maps, core_ids=list(range(NCORES)),
        trace=_trace, **({'tmpdir': _tmpdir} if _tmpdir else {}))
    shards = [res.results[r]['out'] for r in range(NCORES)]
    full = np.concatenate(shards, axis=0)[:N_NODES]
    kernel.last_result = res
    return full.astype(np.float32)



# revision 17
# speedup vs baseline: 1.2164x; 1.2164x over previous
"""HET RelationalAttLayer on 8 trn2 NeuronCores.

Strategy (relation-per-core, all per-core variation is data-driven):
  core r handles relation r (102400 edges each).
  Phase 1: Zext = X @ [W_r | W_r@attn_l_r | W_r@attn_r_r] -> [50176,136] bf16
           (batched: 4 blocks per load DMA / per store DMA).
  Phase 2: 49 super-tiles of 24 dst-sorted 128-edge tiles (8 node blocks):
           per-tile indirect gathers of Zext rows by src and er by dst, then
           BATCHED per super-tile: q = exp(leaky_relu(el+er)), rhs = [q*Z|q],
           one-hot build for all 24 tiles in one broadcast is_equal, 24
           scatter matmuls into packed PSUM banks, one bf16 S store.
  Phase 3: 4 chunked bf16 ReduceScatter(add) collectives, each issued as soon
           as its dst-row range is final (overlaps phase-2 compute);
           batched normalize by denom + bias on the owner core.
Host does index preprocessing only; all float math runs on device.
"""
import numpy as np

N_NODES = 50000
NPAD = 50176
NBLK = NPAD // 128    # 392
N_EDGES = 819200
NUM_RELS = 8
N_HEADS = 4
IN_FEAT = 256
OUT_FEAT = 128
HEAD_DIM = 32
LEAKY = 0.2
NCORES = 8
EPR = N_EDGES // NUM_RELS
ZC = 136              # zext row: 128 Z | 4 el | 4 er
SC = 132              # S row: 128 feat | 4 denom
TPB = 3
T = NBLK * TPB        # 1176
ST = 24               # tiles per super-tile (8 blocks)
NST = T // ST         # 49
BPS = ST // TPB       # 8
GRP = 4               # phase-1 blocks per DMA
CH_ST = [14, 12, 12, 11]
CH_ROWS = [n * BPS * 128 for n in CH_ST]
CH_SHARD = [r // NCORES for r in CH_ROWS]          # 1792,1536,1536,1408
CH_OUT0 = [0, 1792, 3328, 4864]

_CACHE = {}


# ----------------------------------------------------------------- host prep
def _edge_plan(src, dst):
    order = np.argsort(dst, kind='stable')
    src, dst = src[order], dst[order]
    lo = np.searchsorted(dst, np.arange(NBLK) * 128)
    hi = np.searchsorted(dst, np.minimum(np.arange(1, NBLK + 1) * 128, NPAD))
    cnt = hi - lo
    assert cnt.max() <= TPB * 128, f"block overflow: {cnt.max()}"
    t_src = np.zeros((T, 128), np.int32)
    t_dstf = np.full((T, 128), -1.0, np.float32)
    for b in range(NBLK):
        e0, e1 = int(lo[b]), int(hi[b])
        for k in range(TPB):
            a = e0 + k * 128
            bnd = min(a + 128, e1)
            if bnd <= a:
                break
            t = b * TPB + k
            t_src[t, :bnd - a] = src[a:bnd]
            t_dstf[t, :bnd - a] = (dst[a:bnd] - b * 128).astype(np.float32)
    return t_src, t_dstf


def _prep(inputs, conv_weights, attn_l, attn_r, h_bias, row_idx, col_idx):
    import ml_dtypes
    bfd = ml_dtypes.bfloat16
    X = np.asarray(inputs, np.float32)
    W = np.asarray(conv_weights, np.float32)
    al = np.asarray(attn_l, np.float32)
    ar = np.asarray(attn_r, np.float32)
    row = np.asarray(row_idx).astype(np.int64)
    col = np.asarray(col_idx).astype(np.int64)

    Xp = np.zeros((NPAD, IN_FEAT), np.float32)
    Xp[:N_NODES] = X
    xt = Xp.reshape(NBLK, 128, 2, 128).transpose(0, 2, 3, 1).copy()

    blk_of_tile = np.repeat(np.arange(NBLK, dtype=np.int32), TPB)
    per_core = []
    for r in range(NUM_RELS):
        s = row[r * EPR:(r + 1) * EPR].astype(np.int32)
        d = col[r * EPR:(r + 1) * EPR].astype(np.int32)
        src_all, dstf_all = _edge_plan(s, d)
        gdst = np.where(dstf_all >= 0,
                        blk_of_tile[:, None] * 128 + dstf_all.astype(np.int32),
                        0).astype(np.int32)

        Wf = np.concatenate([W[r, h] for h in range(N_HEADS)], axis=1)
        wl = np.stack([W[r, h] @ al[r, h] for h in range(N_HEADS)], axis=1)
        wr = np.stack([W[r, h] @ ar[r, h] for h in range(N_HEADS)], axis=1)
        wext = np.concatenate([Wf, wl, wr], axis=1).astype(np.float32)
        per_core.append(dict(
            wext=wext,
            gsrc=np.ascontiguousarray(src_all.T),               # [128, T] i32
            gdst=np.ascontiguousarray(gdst.T),                  # [128, T] i32
            dstf=np.ascontiguousarray(dstf_all.T).astype(bfd),  # [128, T] bf16
        ))
    consts = dict(
        iota=np.broadcast_to(np.arange(128, dtype=np.float32),
                             (128, 128)).astype(bfd).copy(),
        bias=np.broadcast_to(np.asarray(h_bias, np.float32),
                             (128, 128)).astype(bfd).copy(),
    )
    return xt, per_core, consts


# ------------------------------------------------------------- device program
def _build():
    import concourse.bacc as bacc
    import concourse.tile as tile
    from concourse import mybir
    from concourse.bass import IndirectOffsetOnAxis
    from concourse.bass_types import AP

    f32 = mybir.dt.float32
    bf16 = mybir.dt.bfloat16
    i32 = mybir.dt.int32

    nc = bacc.Bacc("TRN2", target_bir_lowering=False, debug=False,
                   num_devices=NCORES)

    xt = nc.dram_tensor("xt", [NBLK, 2, 128, 128], f32, kind="ExternalInput")
    wext = nc.dram_tensor("wext", [IN_FEAT, ZC], f32, kind="ExternalInput")
    gsrc = nc.dram_tensor("gsrc", [128, T], i32, kind="ExternalInput")
    gdst = nc.dram_tensor("gdst", [128, T], i32, kind="ExternalInput")
    dstf = nc.dram_tensor("dstf", [128, T], bf16, kind="ExternalInput")
    iota = nc.dram_tensor("iota", [128, 128], bf16, kind="ExternalInput")
    bias = nc.dram_tensor("bias", [128, 128], bf16, kind="ExternalInput")
    out = nc.dram_tensor("out", [sum(CH_SHARD), OUT_FEAT], f32,
                         kind="ExternalOutput")

    with tile.TileContext(nc) as tc:
        with tc.tile_pool(name="dram", bufs=1, space="DRAM") as dram, \
             tc.tile_pool(name="consts", bufs=1) as cpool, \
             tc.tile_pool(name="ph1", bufs=3) as ph1, \
             tc.tile_pool(name="ph1z", bufs=3) as ph1z, \
             tc.tile_pool(name="ph1p", bufs=2, space="PSUM") as ph1p, \
             tc.tile_pool(name="gat", bufs=2) as gat, \
             tc.tile_pool(name="rhp", bufs=2) as rhp, \
             tc.tile_pool(name="edg", bufs=3) as edg, \
             tc.tile_pool(name="ohp", bufs=2) as ohp, \
             tc.tile_pool(name="ph2p", bufs=6, space="PSUM") as ph2p, \
             tc.tile_pool(name="outp", bufs=4) as outp, \
             tc.tile_pool(name="nrm", bufs=3) as nrm:

            zext = dram.tile([NPAD, ZC], bf16)
            ern = dram.tile([NPAD, 4], bf16)
            s_acc = [dram.tile([CH_ROWS[i], SC], bf16, name=f"s_acc{i}")
                     for i in range(4)]
            s_red = [dram.tile([CH_SHARD[i], SC], bf16, name=f"s_red{i}")
                     for i in range(4)]

            w_sb = cpool.tile([128, 2 * ZC], f32, tag="wext")
            wap = wext[:]
            nc.sync.dma_start(
                w_sb[:], AP(wap.tensor, 0, [[ZC, 128], [128 * ZC, 2], [1, ZC]]))
            iota_sb = cpool.tile([128, 128], bf16, tag="iota")
            nc.sync.dma_start(iota_sb[:], iota[:])
            bias_sb = cpool.tile([128, 128], bf16, tag="bias")
            nc.sync.dma_start(bias_sb[:], bias[:])
            gs_sb = cpool.tile([128, T], i32, tag="gs")
            nc.sync.dma_start(gs_sb[:], gsrc[:])
            gd_sb = cpool.tile([128, T], i32, tag="gd")
            nc.sync.dma_start(gd_sb[:], gdst[:])
            df_sb = cpool.tile([128, T], bf16, tag="df")
            nc.sync.dma_start(df_sb[:], dstf[:])

            xa = xt[:]
            za = zext[:]
            ea = ern[:]

            # ---------------- Phase 1: Zext GEMM ----------------
            for g in range(NBLK // GRP):
                lh = ph1.tile([128, GRP * 2 * 128], f32, tag="lh")
                nc.sync.dma_start(
                    lh[:], AP(xa.tensor, g * GRP * 2 * 128 * 128,
                              [[128, 128], [2 * 128 * 128, GRP],
                               [128 * 128, 2], [1, 128]]))
                zr = ph1z.tile([128, GRP * ZC], bf16, tag="zr")
                for bb in range(GRP):
                    ps = ph1p.tile([128, ZC], f32, space="PSUM", tag="zps")
                    nc.tensor.matmul(out=ps[:],
                                     lhsT=lh[:, bb * 256:bb * 256 + 128],
                                     rhs=w_sb[:, 0:ZC],
                                     start=True, stop=False)
                    nc.tensor.matmul(out=ps[:],
                                     lhsT=lh[:, bb * 256 + 128:bb * 256 + 256],
                                     rhs=w_sb[:, ZC:2 * ZC],
                                     start=False, stop=True)
                    nc.scalar.copy(zr[:, bb * ZC:(bb + 1) * ZC], ps[:])
                zrap = zr[:]
                nc.sync.dma_start(
                    AP(za.tensor, za.offset + g * GRP * 128 * ZC,
                       [[ZC, 128], [128 * ZC, GRP], [1, ZC]]),
                    AP(zrap.tensor, zrap.offset,
                       [zrap.ap[0], [ZC, GRP], [1, ZC]]))
                nc.sync.dma_start(
                    AP(ea.tensor, ea.offset + g * GRP * 128 * 4,
                       [[4, 128], [128 * 4, GRP], [1, 4]]),
                    AP(zrap.tensor, zrap.offset + 132,
                       [zrap.ap[0], [ZC, GRP], [1, 4]]))

            # ---------------- Phase 2: edges ----------------
            ci = 0
            st_hi = CH_ST[0]
            ch_st0 = 0
            for s in range(NST):
                if s >= st_hi:
                    ch_st0 = st_hi
                    ci += 1
                    st_hi += CH_ST[ci]
                G = gat.tile([128, ST * ZC], bf16, tag="G")
                erg = edg.tile([128, ST * 4], bf16, tag="erg")
                for k in range(ST):
                    t = s * ST + k
                    nc.gpsimd.indirect_dma_start(
                        out=G[:, k * ZC:(k + 1) * ZC], out_offset=None,
                        in_=zext[:],
                        in_offset=IndirectOffsetOnAxis(
                            ap=gs_sb[:, t:t + 1], axis=0))
                    nc.gpsimd.indirect_dma_start(
                        out=erg[:, k * 4:(k + 1) * 4], out_offset=None,
                        in_=ern[:],
                        in_offset=IndirectOffsetOnAxis(
                            ap=gd_sb[:, t:t + 1], axis=0))
                gap = G[:]
                el_view = AP(gap.tensor, gap.offset + 128,
                             [gap.ap[0], [ZC, ST], [1, 4]])
                q0 = edg.tile([128, ST * 4], bf16, tag="q0")
                nc.vector.tensor_tensor(q0[:], el_view, erg[:],
                                        op=mybir.AluOpType.add)
                qs = edg.tile([128, ST * 4], bf16, tag="qs")
                nc.vector.tensor_scalar(qs[:], q0[:], LEAKY, None,
                                        op0=mybir.AluOpType.mult)
                q1 = edg.tile([128, ST * 4], bf16, tag="q1")
                nc.vector.tensor_tensor(q1[:], q0[:], qs[:],
                                        op=mybir.AluOpType.max)
                q = edg.tile([128, ST * 4], bf16, tag="q")
                nc.scalar.activation(q[:], q1[:],
                                     mybir.ActivationFunctionType.Exp)
                rhs = rhp.tile([128, ST * SC], bf16, tag="rhs")
                rap = rhs[:]
                qap = q[:]
                nc.vector.tensor_tensor(
                    AP(rap.tensor, rap.offset, [rap.ap[0], [SC, ST], [1, 128]]),
                    AP(gap.tensor, gap.offset, [gap.ap[0], [ZC, ST], [1, 128]]),
                    AP(qap.tensor, qap.offset,
                       [qap.ap[0], [4, ST], [1, 4], [0, HEAD_DIM]]),
                    op=mybir.AluOpType.mult)
                nc.vector.tensor_copy(
                    AP(rap.tensor, rap.offset + 128,
                       [rap.ap[0], [SC, ST], [1, 4]]),
                    qap)

                oh = ohp.tile([128, ST * 128], bf16, tag="oh")
                iap = iota_sb[:]
                dap = df_sb[:]
                nc.vector.tensor_tensor(
                    oh[:],
                    AP(iap.tensor, iap.offset, [iap.ap[0], [0, ST], [1, 128]]),
                    AP(dap.tensor, dap.offset + s * ST,
                       [dap.ap[0], [1, ST], [0, 128]]),
                    op=mybir.AluOpType.is_equal)

                pt = [ph2p.tile([128, 3 * SC], f32, space="PSUM", tag="sps",
                                name=f"pt{j}")
                      for j in range(3)]
                for k in range(ST):
                    blk = k // TPB
                    dst = pt[blk // 3][:, (blk % 3) * SC:(blk % 3 + 1) * SC]
                    nc.tensor.matmul(
                        out=dst, lhsT=oh[:, k * 128:(k + 1) * 128],
                        rhs=rhs[:, k * SC:(k + 1) * SC],
                        start=(k % TPB == 0), stop=(k % TPB == TPB - 1))
                sow = outp.tile([128, BPS * SC], bf16, tag="sow")
                for j in range(3):
                    w = (3 * SC) if j < 2 else (2 * SC)
                    nc.scalar.copy(sow[:, j * 3 * SC:j * 3 * SC + w],
                                   pt[j][:, :w])
                sa = s_acc[ci][:]
                soap = sow[:]
                nc.sync.dma_start(
                    AP(sa.tensor, sa.offset + (s - ch_st0) * BPS * 128 * SC,
                       [[SC, 128], [128 * SC, BPS], [1, SC]]),
                    AP(soap.tensor, soap.offset,
                       [soap.ap[0], [SC, BPS], [1, SC]]))
                if s == st_hi - 1:
                    nc.gpsimd.collective_compute(
                        "ReduceScatter", mybir.AluOpType.add,
                        replica_groups=[list(range(NCORES))],
                        ins=[s_acc[ci][:].opt()], outs=[s_red[ci][:].opt()])

            # ---------------- Phase 3: normalize ----------------
            oa = out[:]
            for i in range(4):
                sr = s_red[i][:]
                nblk_i = CH_SHARD[i] // 128
                g0 = 0
                while g0 < nblk_i:
                    nb = min(7, nblk_i - g0)
                    stt = nrm.tile([128, 7 * SC], bf16, tag="stt")
                    nc.sync.dma_start(
                        stt[:, :nb * SC],
                        AP(sr.tensor, sr.offset + g0 * 128 * SC,
                           [[SC, 128], [128 * SC, nb], [1, SC]]))
                    sta = stt[:]
                    dg = nrm.tile([128, 7 * 4], bf16, tag="dg")
                    nc.vector.tensor_scalar(
                        dg[:, :nb * 4],
                        AP(sta.tensor, sta.offset + 128,
                           [sta.ap[0], [SC, nb], [1, 4]]),
                        1e-30, None, op0=mybir.AluOpType.max)
                    rc = nrm.tile([128, 7 * 4], bf16, tag="rc")
                    with nc.allow_low_precision(reason="bf16 recip ok at tol"):
                        nc.vector.reciprocal(rc[:, :nb * 4], dg[:, :nb * 4])
                    rca = rc[:]
                    ot = nrm.tile([128, 7 * 128], bf16, tag="ot")
                    nc.vector.tensor_tensor(
                        ot[:, :nb * 128],
                        AP(sta.tensor, sta.offset,
                           [sta.ap[0], [SC, nb], [1, 128]]),
                        AP(rca.tensor, rca.offset,
                           [rca.ap[0], [4, nb], [1, 4], [0, HEAD_DIM]]),
                        op=mybir.AluOpType.mult)
                    ot2 = nrm.tile([128, 7 * 128], f32, tag="ot2")
                    bap = bias_sb[:]
                    nc.vector.tensor_tensor(
                        ot2[:, :nb * 128],
                        ot[:, :nb * 128],
                        AP(bap.tensor, bap.offset,
                           [bap.ap[0], [0, nb], [1, 128]]),
                        op=mybir.AluOpType.add)
                    o2a = ot2[:]
                    nc.sync.dma_start(
                        AP(oa.tensor,
                           (CH_OUT0[i] + g0 * 128) * OUT_FEAT,
                           [[OUT_FEAT, 128], [128 * OUT_FEAT, nb],
                            [1, OUT_FEAT]]),
                        AP(o2a.tensor, o2a.offset,
                           [o2a.ap[0], [OUT_FEAT, nb], [1, OUT_FEAT]]))
                    g0 += nb

    nc.compile()
    return nc


# ------------------------------------------------------------------- entry
def kernel(inputs, conv_weights, attn_l, attn_r, h_bias, row_idx, col_idx,
           _trace=False, _tmpdir=None):
    xt, per_core, consts = _prep(inputs, conv_weights, attn_l, attn_r,
                                 h_bias, row_idx, col_idx)
    if 'nc' not in _CACHE:
        _CACHE['nc'] = _build()
    nc = _CACHE['nc']

    in_maps = []
    for r in range(NCORES):
        pc = per_core[r]
        in_maps.append(dict(
            xt=xt, wext=pc['wext'],
            gsrc=pc['gsrc'], gdst=pc['gdst'], dstf=pc['dstf'],
            iota=consts['iota'], bias=consts['bias'],
        ))

    from concourse import bass_utils
    res = bass_utils.run_bass_kernel_spmd(
        nc, in_maps, core_ids=list(range(NCORES)),
        trace=_trace, **({'tmpdir': _tmpdir} if _tmpdir else {}))
    full = np.empty((NPAD, OUT_FEAT), np.float32)
    ch_row0 = np.cumsum([0] + CH_ROWS[:-1])
    for c in range(NCORES):
        oc = np.asarray(res.results[c]['out'], np.float32)
        for i in range(4):
            gl0 = ch_row0[i] + c * CH_SHARD[i]
            full[gl0:gl0 + CH_SHARD[i]] = (
                oc[CH_OUT0[i]:CH_OUT0[i] + CH_SHARD[i]])
    kernel.last_result = res
    return full[:N_NODES]


# revision 19
# speedup vs baseline: 1.2358x; 1.0159x over previous
"""HET RelationalAttLayer on 8 trn2 NeuronCores.

Strategy (relation-per-core, all per-core variation is data-driven):
  core r handles relation r (102400 edges each).
  Phase 1: Zext = X @ [W_r | W_r@attn_l_r | W_r@attn_r_r] -> [50176,136] bf16
           (batched: 4 blocks per load DMA / per store DMA).
  Phase 2: 49 super-tiles of 24 dst-sorted 128-edge tiles (8 node blocks):
           per-tile indirect gathers of Zext rows by src and er by dst, then
           BATCHED per super-tile: q = exp(leaky_relu(el+er)), rhs = [q*Z|q],
           one-hot build for all 24 tiles in one broadcast is_equal, 24
           scatter matmuls into packed PSUM banks, one bf16 S store.
  Phase 3: 4 chunked bf16 ReduceScatter(add) collectives, each issued as soon
           as its dst-row range is final (overlaps phase-2 compute);
           batched normalize by denom + bias on the owner core.
Host does index preprocessing only; all float math runs on device.
"""
import numpy as np

N_NODES = 50000
NPAD = 50176
NBLK = NPAD // 128    # 392
N_EDGES = 819200
NUM_RELS = 8
N_HEADS = 4
IN_FEAT = 256
OUT_FEAT = 128
HEAD_DIM = 32
LEAKY = 0.2
NCORES = 8
EPR = N_EDGES // NUM_RELS
ZC = 136              # zext row: 128 Z | 4 el | 4 er
SC = 132              # S row: 128 feat | 4 denom
TPB = 3
T = NBLK * TPB        # 1176
ST = 24               # tiles per super-tile (8 blocks)
NST = T // ST         # 49
BPS = ST // TPB       # 8
GRP = 4               # phase-1 blocks per DMA
CH_ST = [14, 12, 12, 11]
CH_ROWS = [n * BPS * 128 for n in CH_ST]
CH_SHARD = [r // NCORES for r in CH_ROWS]          # 1792,1536,1536,1408
CH_OUT0 = [0, 1792, 3328, 4864]

_CACHE = {}


# ----------------------------------------------------------------- host prep
def _edge_plan(src, dst):
    order = np.argsort(dst, kind='stable')
    src, dst = src[order], dst[order]
    lo = np.searchsorted(dst, np.arange(NBLK) * 128)
    hi = np.searchsorted(dst, np.minimum(np.arange(1, NBLK + 1) * 128, NPAD))
    cnt = hi - lo
    assert cnt.max() <= TPB * 128, f"block overflow: {cnt.max()}"
    t_src = np.zeros((T, 128), np.int32)
    t_dstf = np.full((T, 128), -1.0, np.float32)
    for b in range(NBLK):
        e0, e1 = int(lo[b]), int(hi[b])
        for k in range(TPB):
            a = e0 + k * 128
            bnd = min(a + 128, e1)
            if bnd <= a:
                break
            t = b * TPB + k
            t_src[t, :bnd - a] = src[a:bnd]
            t_dstf[t, :bnd - a] = (dst[a:bnd] - b * 128).astype(np.float32)
    return t_src, t_dstf


def _prep(inputs, conv_weights, attn_l, attn_r, h_bias, row_idx, col_idx):
    import ml_dtypes
    bfd = ml_dtypes.bfloat16
    X = np.asarray(inputs, np.float32)
    W = np.asarray(conv_weights, np.float32)
    al = np.asarray(attn_l, np.float32)
    ar = np.asarray(attn_r, np.float32)
    row = np.asarray(row_idx).astype(np.int64)
    col = np.asarray(col_idx).astype(np.int64)

    Xp = np.zeros((NPAD, IN_FEAT), np.float32)
    Xp[:N_NODES] = X
    xt = Xp.reshape(NBLK, 128, 2, 128).transpose(0, 2, 3, 1).copy()

    blk_of_tile = np.repeat(np.arange(NBLK, dtype=np.int32), TPB)
    per_core = []
    for r in range(NUM_RELS):
        s = row[r * EPR:(r + 1) * EPR].astype(np.int32)
        d = col[r * EPR:(r + 1) * EPR].astype(np.int32)
        src_all, dstf_all = _edge_plan(s, d)
        gdst = np.where(dstf_all >= 0,
                        blk_of_tile[:, None] * 128 + dstf_all.astype(np.int32),
                        0).astype(np.int32)

        Wf = np.concatenate([W[r, h] for h in range(N_HEADS)], axis=1)
        wl = np.stack([W[r, h] @ al[r, h] for h in range(N_HEADS)], axis=1)
        wr = np.stack([W[r, h] @ ar[r, h] for h in range(N_HEADS)], axis=1)
        wext = np.concatenate([Wf, wl, wr], axis=1).astype(np.float32)
        per_core.append(dict(
            wext=wext,
            gsrc=np.ascontiguousarray(src_all.T),               # [128, T] i32
            gdst=np.ascontiguousarray(gdst.T),                  # [128, T] i32
            dstf=np.ascontiguousarray(dstf_all.T).astype(bfd),  # [128, T] bf16
        ))
    consts = dict(
        iota=np.broadcast_to(np.arange(128, dtype=np.float32),
                             (128, 128)).astype(bfd).copy(),
        bias=np.broadcast_to(np.asarray(h_bias, np.float32),
                             (128, 128)).astype(bfd).copy(),
    )
    return xt, per_core, consts


# ------------------------------------------------------------- device program
def _build():
    import concourse.bacc as bacc
    import concourse.tile as tile
    from concourse import mybir
    from concourse.bass import IndirectOffsetOnAxis
    from concourse.bass_types import AP

    f32 = mybir.dt.float32
    bf16 = mybir.dt.bfloat16
    i32 = mybir.dt.int32

    nc = bacc.Bacc("TRN2", target_bir_lowering=False, debug=False,
                   num_devices=NCORES)

    xt = nc.dram_tensor("xt", [NBLK, 2, 128, 128], f32, kind="ExternalInput")
    wext = nc.dram_tensor("wext", [IN_FEAT, ZC], f32, kind="ExternalInput")
    gsrc = nc.dram_tensor("gsrc", [128, T], i32, kind="ExternalInput")
    gdst = nc.dram_tensor("gdst", [128, T], i32, kind="ExternalInput")
    dstf = nc.dram_tensor("dstf", [128, T], bf16, kind="ExternalInput")
    iota = nc.dram_tensor("iota", [128, 128], bf16, kind="ExternalInput")
    bias = nc.dram_tensor("bias", [128, 128], bf16, kind="ExternalInput")
    out = nc.dram_tensor("out", [sum(CH_SHARD), OUT_FEAT], f32,
                         kind="ExternalOutput")

    with tile.TileContext(nc) as tc:
        with tc.tile_pool(name="dram", bufs=1, space="DRAM") as dram, \
             tc.tile_pool(name="consts", bufs=1) as cpool, \
             tc.tile_pool(name="ph1", bufs=3) as ph1, \
             tc.tile_pool(name="ph1z", bufs=3) as ph1z, \
             tc.tile_pool(name="ph1p", bufs=2, space="PSUM") as ph1p, \
             tc.tile_pool(name="gat", bufs=2) as gat, \
             tc.tile_pool(name="rhp", bufs=2) as rhp, \
             tc.tile_pool(name="edg", bufs=3) as edg, \
             tc.tile_pool(name="ohp", bufs=2) as ohp, \
             tc.tile_pool(name="ph2p", bufs=6, space="PSUM") as ph2p, \
             tc.tile_pool(name="outp", bufs=4) as outp, \
             tc.tile_pool(name="nrm", bufs=3) as nrm:

            zext = dram.tile([NPAD, ZC], bf16)
            ern = dram.tile([NPAD, 4], bf16)
            s_acc = [dram.tile([CH_ROWS[i], SC], bf16, name=f"s_acc{i}")
                     for i in range(4)]
            s_red = [dram.tile([CH_SHARD[i], SC], bf16, name=f"s_red{i}")
                     for i in range(4)]

            w_sb = cpool.tile([128, 2 * ZC], f32, tag="wext")
            wap = wext[:]
            nc.sync.dma_start(
                w_sb[:], AP(wap.tensor, 0, [[ZC, 128], [128 * ZC, 2], [1, ZC]]))
            iota_sb = cpool.tile([128, 128], bf16, tag="iota")
            nc.sync.dma_start(iota_sb[:], iota[:])
            bias_sb = cpool.tile([128, 128], bf16, tag="bias")
            nc.sync.dma_start(bias_sb[:], bias[:])
            gs_sb = cpool.tile([128, T], i32, tag="gs")
            nc.sync.dma_start(gs_sb[:], gsrc[:])
            gd_sb = cpool.tile([128, T], i32, tag="gd")
            nc.sync.dma_start(gd_sb[:], gdst[:])
            df_sb = cpool.tile([128, T], bf16, tag="df")
            nc.sync.dma_start(df_sb[:], dstf[:])

            xa = xt[:]
            za = zext[:]
            ea = ern[:]

            # ---------------- Phase 1: Zext GEMM ----------------
            for g in range(NBLK // GRP):
                lh = ph1.tile([128, GRP * 2 * 128], f32, tag="lh")
                nc.sync.dma_start(
                    lh[:], AP(xa.tensor, g * GRP * 2 * 128 * 128,
                              [[128, 128], [2 * 128 * 128, GRP],
                               [128 * 128, 2], [1, 128]]))
                zr = ph1z.tile([128, GRP * ZC], bf16, tag="zr")
                for bb in range(GRP):
                    ps = ph1p.tile([128, ZC], f32, space="PSUM", tag="zps")
                    nc.tensor.matmul(out=ps[:],
                                     lhsT=lh[:, bb * 256:bb * 256 + 128],
                                     rhs=w_sb[:, 0:ZC],
                                     start=True, stop=False)
                    nc.tensor.matmul(out=ps[:],
                                     lhsT=lh[:, bb * 256 + 128:bb * 256 + 256],
                                     rhs=w_sb[:, ZC:2 * ZC],
                                     start=False, stop=True)
                    nc.scalar.copy(zr[:, bb * ZC:(bb + 1) * ZC], ps[:])
                zrap = zr[:]
                nc.sync.dma_start(
                    AP(za.tensor, za.offset + g * GRP * 128 * ZC,
                       [[ZC, 128], [128 * ZC, GRP], [1, ZC]]),
                    AP(zrap.tensor, zrap.offset,
                       [zrap.ap[0], [ZC, GRP], [1, ZC]]))
                nc.sync.dma_start(
                    AP(ea.tensor, ea.offset + g * GRP * 128 * 4,
                       [[4, 128], [128 * 4, GRP], [1, 4]]),
                    AP(zrap.tensor, zrap.offset + 132,
                       [zrap.ap[0], [ZC, GRP], [1, 4]]))

            # ---------------- Phase 2: edges ----------------
            ci = 0
            st_hi = CH_ST[0]
            ch_st0 = 0
            for s in range(NST):
                if s >= st_hi:
                    ch_st0 = st_hi
                    ci += 1
                    st_hi += CH_ST[ci]
                G = gat.tile([128, ST * ZC], bf16, tag="G")
                erg = edg.tile([128, ST * 4], bf16, tag="erg")
                for k in range(ST):
                    t = s * ST + k
                    nc.gpsimd.indirect_dma_start(
                        out=G[:, k * ZC:(k + 1) * ZC], out_offset=None,
                        in_=zext[:],
                        in_offset=IndirectOffsetOnAxis(
                            ap=gs_sb[:, t:t + 1], axis=0))
                    nc.gpsimd.indirect_dma_start(
                        out=erg[:, k * 4:(k + 1) * 4], out_offset=None,
                        in_=ern[:],
                        in_offset=IndirectOffsetOnAxis(
                            ap=gd_sb[:, t:t + 1], axis=0))
                gap = G[:]
                el_view = AP(gap.tensor, gap.offset + 128,
                             [gap.ap[0], [ZC, ST], [1, 4]])
                q0 = edg.tile([128, ST * 4], bf16, tag="q0")
                nc.vector.tensor_tensor(q0[:], el_view, erg[:],
                                        op=mybir.AluOpType.add)
                qs = edg.tile([128, ST * 4], bf16, tag="qs")
                nc.vector.tensor_scalar(qs[:], q0[:], LEAKY, None,
                                        op0=mybir.AluOpType.mult)
                q1 = edg.tile([128, ST * 4], bf16, tag="q1")
                nc.vector.tensor_tensor(q1[:], q0[:], qs[:],
                                        op=mybir.AluOpType.max)
                q = edg.tile([128, ST * 4], bf16, tag="q")
                nc.scalar.activation(q[:], q1[:],
                                     mybir.ActivationFunctionType.Exp)
                rhs = rhp.tile([128, ST * SC], bf16, tag="rhs")
                rap = rhs[:]
                qap = q[:]
                nc.vector.tensor_tensor(
                    AP(rap.tensor, rap.offset, [rap.ap[0], [SC, ST], [1, 128]]),
                    AP(gap.tensor, gap.offset, [gap.ap[0], [ZC, ST], [1, 128]]),
                    AP(qap.tensor, qap.offset,
                       [qap.ap[0], [4, ST], [1, 4], [0, HEAD_DIM]]),
                    op=mybir.AluOpType.mult)
                nc.vector.tensor_copy(
                    AP(rap.tensor, rap.offset + 128,
                       [rap.ap[0], [SC, ST], [1, 4]]),
                    qap)

                oh = ohp.tile([128, ST * 128], bf16, tag="oh")
                iap = iota_sb[:]
                dap = df_sb[:]
                nc.vector.tensor_tensor(
                    oh[:],
                    AP(iap.tensor, iap.offset, [iap.ap[0], [0, ST], [1, 128]]),
                    AP(dap.tensor, dap.offset + s * ST,
                       [dap.ap[0], [1, ST], [0, 128]]),
                    op=mybir.AluOpType.is_equal)

                pt = [ph2p.tile([128, 3 * SC], f32, space="PSUM", tag="sps",
                                name=f"pt{j}")
                      for j in range(3)]
                for k in range(ST):
                    blk = k // TPB
                    dst = pt[blk // 3][:, (blk % 3) * SC:(blk % 3 + 1) * SC]
                    nc.tensor.matmul(
                        out=dst, lhsT=oh[:, k * 128:(k + 1) * 128],
                        rhs=rhs[:, k * SC:(k + 1) * SC],
                        start=(k % TPB == 0), stop=(k % TPB == TPB - 1))
                sow = outp.tile([128, BPS * SC], bf16, tag="sow")
                for j in range(3):
                    w = (3 * SC) if j < 2 else (2 * SC)
                    nc.scalar.copy(sow[:, j * 3 * SC:j * 3 * SC + w],
                                   pt[j][:, :w])
                sa = s_acc[ci][:]
                soap = sow[:]
                nc.sync.dma_start(
                    AP(sa.tensor, sa.offset + (s - ch_st0) * BPS * 128 * SC,
                       [[SC, 128], [128 * SC, BPS], [1, SC]]),
                    AP(soap.tensor, soap.offset,
                       [soap.ap[0], [SC, BPS], [1, SC]]))
                if s == st_hi - 1:
                    nc.gpsimd.collective_compute(
                        "ReduceScatter", mybir.AluOpType.add,
                        replica_groups=[list(range(NCORES))],
                        ins=[s_acc[ci][:].opt()], outs=[s_red[ci][:].opt()])

            # ---------------- Phase 3: normalize ----------------
            oa = out[:]
            for i in range(4):
                sr = s_red[i][:]
                nblk_i = CH_SHARD[i] // 128
                g0 = 0
                while g0 < nblk_i:
                    nb = min(7, nblk_i - g0)
                    stt = nrm.tile([128, 7 * SC], bf16, tag="stt")
                    nc.sync.dma_start(
                        stt[:, :nb * SC],
                        AP(sr.tensor, sr.offset + g0 * 128 * SC,
                           [[SC, 128], [128 * SC, nb], [1, SC]]))
                    sta = stt[:]
                    dg = nrm.tile([128, 7 * 4], bf16, tag="dg")
                    nc.vector.tensor_scalar(
                        dg[:, :nb * 4],
                        AP(sta.tensor, sta.offset + 128,
                           [sta.ap[0], [SC, nb], [1, 4]]),
                        1e-30, None, op0=mybir.AluOpType.max)
                    rc = nrm.tile([128, 7 * 4], bf16, tag="rc")
                    with nc.allow_low_precision(reason="bf16 recip ok at tol"):
                        nc.vector.reciprocal(rc[:, :nb * 4], dg[:, :nb * 4])
                    rca = rc[:]
                    ot = nrm.tile([128, 7 * 128], bf16, tag="ot")
                    nc.vector.tensor_tensor(
                        ot[:, :nb * 128],
                        AP(sta.tensor, sta.offset,
                           [sta.ap[0], [SC, nb], [1, 128]]),
                        AP(rca.tensor, rca.offset,
                           [rca.ap[0], [4, nb], [1, 4], [0, HEAD_DIM]]),
                        op=mybir.AluOpType.mult)
                    ot2 = nrm.tile([128, 7 * 128], f32, tag="ot2")
                    bap = bias_sb[:]
                    nc.vector.tensor_tensor(
                        ot2[:, :nb * 128],
                        ot[:, :nb * 128],
                        AP(bap.tensor, bap.offset,
                           [bap.ap[0], [0, nb], [1, 128]]),
                        op=mybir.AluOpType.add)
                    o2a = ot2[:]
                    nc.sync.dma_start(
                        AP(oa.tensor,
                           (CH_OUT0[i] + g0 * 128) * OUT_FEAT,
                           [[OUT_FEAT, 128], [128 * OUT_FEAT, nb],
                            [1, OUT_FEAT]]),
                        AP(o2a.tensor, o2a.offset,
                           [o2a.ap[0], [OUT_FEAT, nb], [1, OUT_FEAT]]))
                    g0 += nb

    nc.compile()
    return nc


# ------------------------------------------------------------------- entry
def kernel(inputs, conv_weights, attn_l, attn_r, h_bias, row_idx, col_idx,
           _trace=False, _tmpdir=None):
    xt, per_core, consts = _prep(inputs, conv_weights, attn_l, attn_r,
                                 h_bias, row_idx, col_idx)
    if 'nc' not in _CACHE:
        _CACHE['nc'] = _build()
    nc = _CACHE['nc']

    in_maps = []
    for r in range(NCORES):
        pc = per_core[r]
        in_maps.append(dict(
            xt=xt, wext=pc['wext'],
            gsrc=pc['gsrc'], gdst=pc['gdst'], dstf=pc['dstf'],
            iota=consts['iota'], bias=consts['bias'],
        ))

    from concourse import bass_utils
    res = bass_utils.run_bass_kernel_spmd(
        nc, in_maps, core_ids=list(range(NCORES)),
        trace=_trace, **({'tmpdir': _tmpdir} if _tmpdir else {}))
    full = np.empty((NPAD, OUT_FEAT), np.float32)
    ch_row0 = np.cumsum([0] + CH_ROWS[:-1])
    for c in range(NCORES):
        oc = np.asarray(res.results[c]['out'], np.float32)
        for i in range(4):
            gl0 = ch_row0[i] + c * CH_SHARD[i]
            full[gl0:gl0 + CH_SHARD[i]] = (
                oc[CH_OUT0[i]:CH_OUT0[i] + CH_SHARD[i]])
    kernel.last_result = res
    return full[:N_NODES]


# revision 22
# speedup vs baseline: 1.2407x; 1.0040x over previous
"""HET RelationalAttLayer on 8 trn2 NeuronCores.

Strategy (relation-per-core, all per-core variation is data-driven):
  core r handles relation r (102400 edges each).
  Phase 1: Zext = X @ [W_r | W_r@attn_l_r | W_r@attn_r_r] -> [50176,136] bf16
           (batched: 4 blocks per load DMA / per store DMA).
  Phase 2: 49 super-tiles of 24 dst-sorted 128-edge tiles (8 node blocks):
           per-tile indirect gathers of Zext rows by src and er by dst, then
           BATCHED per super-tile: q = exp(leaky_relu(el+er)), rhs = [q*Z|q],
           one-hot build for all 24 tiles in one broadcast is_equal, 24
           scatter matmuls into packed PSUM banks, one bf16 S store.
  Phase 3: 4 chunked bf16 ReduceScatter(add) collectives, each issued as soon
           as its dst-row range is final (overlaps phase-2 compute);
           batched normalize by denom + bias on the owner core.
Host does index preprocessing only; all float math runs on device.
"""
import numpy as np

N_NODES = 50000
NPAD = 50176
NBLK = NPAD // 128    # 392
N_EDGES = 819200
NUM_RELS = 8
N_HEADS = 4
IN_FEAT = 256
OUT_FEAT = 128
HEAD_DIM = 32
LEAKY = 0.2
NCORES = 8
EPR = N_EDGES // NUM_RELS
ZC = 136              # zext row: 128 Z | 4 el | 4 er
SC = 132              # S row: 128 feat | 4 denom
TPB = 3
T = NBLK * TPB        # 1176
ST = 24               # tiles per super-tile (8 blocks)
NST = T // ST         # 49
BPS = ST // TPB       # 8
GRP = 4               # phase-1 blocks per DMA
CH_ST = [14, 12, 12, 11]
CH_ROWS = [n * BPS * 128 for n in CH_ST]
CH_SHARD = [r // NCORES for r in CH_ROWS]          # 1792,1536,1536,1408
CH_OUT0 = [0, 1792, 3328, 4864]

_CACHE = {}


# ----------------------------------------------------------------- host prep
def _edge_plan(src, dst):
    order = np.argsort(dst, kind='stable')
    src, dst = src[order], dst[order]
    lo = np.searchsorted(dst, np.arange(NBLK) * 128)
    hi = np.searchsorted(dst, np.minimum(np.arange(1, NBLK + 1) * 128, NPAD))
    cnt = hi - lo
    assert cnt.max() <= TPB * 128, f"block overflow: {cnt.max()}"
    t_src = np.zeros((T, 128), np.int32)
    t_dstf = np.full((T, 128), -1.0, np.float32)
    for b in range(NBLK):
        e0, e1 = int(lo[b]), int(hi[b])
        for k in range(TPB):
            a = e0 + k * 128
            bnd = min(a + 128, e1)
            if bnd <= a:
                break
            t = b * TPB + k
            t_src[t, :bnd - a] = src[a:bnd]
            t_dstf[t, :bnd - a] = (dst[a:bnd] - b * 128).astype(np.float32)
    return t_src, t_dstf


def _prep(inputs, conv_weights, attn_l, attn_r, h_bias, row_idx, col_idx):
    import ml_dtypes
    bfd = ml_dtypes.bfloat16
    X = np.asarray(inputs, np.float32)
    W = np.asarray(conv_weights, np.float32)
    al = np.asarray(attn_l, np.float32)
    ar = np.asarray(attn_r, np.float32)
    row = np.asarray(row_idx).astype(np.int64)
    col = np.asarray(col_idx).astype(np.int64)

    Xp = np.zeros((NPAD, IN_FEAT), np.float32)
    Xp[:N_NODES] = X
    xt = Xp.reshape(NBLK, 128, 2, 128).transpose(0, 2, 3, 1).copy()

    blk_of_tile = np.repeat(np.arange(NBLK, dtype=np.int32), TPB)
    per_core = []
    for r in range(NUM_RELS):
        s = row[r * EPR:(r + 1) * EPR].astype(np.int32)
        d = col[r * EPR:(r + 1) * EPR].astype(np.int32)
        src_all, dstf_all = _edge_plan(s, d)
        gdst = np.where(dstf_all >= 0,
                        blk_of_tile[:, None] * 128 + dstf_all.astype(np.int32),
                        0).astype(np.int32)

        Wf = np.concatenate([W[r, h] for h in range(N_HEADS)], axis=1)
        wl = np.stack([W[r, h] @ al[r, h] for h in range(N_HEADS)], axis=1)
        wr = np.stack([W[r, h] @ ar[r, h] for h in range(N_HEADS)], axis=1)
        wext = np.concatenate([Wf, wl, wr], axis=1).astype(np.float32)
        per_core.append(dict(
            wext=wext,
            gsrc=np.ascontiguousarray(src_all.T),               # [128, T] i32
            gdst=np.ascontiguousarray(gdst.T),                  # [128, T] i32
            dstf=np.ascontiguousarray(dstf_all.T).astype(bfd),  # [128, T] bf16
        ))
    consts = dict(
        iota=np.broadcast_to(np.arange(128, dtype=np.float32),
                             (128, 128)).astype(bfd).copy(),
        bias=np.broadcast_to(np.asarray(h_bias, np.float32),
                             (128, 128)).astype(bfd).copy(),
    )
    return xt, per_core, consts


# ------------------------------------------------------------- device program
def _build():
    import concourse.bacc as bacc
    import concourse.tile as tile
    from concourse import mybir
    from concourse.bass import IndirectOffsetOnAxis
    from concourse.bass_types import AP

    f32 = mybir.dt.float32
    bf16 = mybir.dt.bfloat16
    i32 = mybir.dt.int32

    nc = bacc.Bacc("TRN2", target_bir_lowering=False, debug=False,
                   num_devices=NCORES)

    xt = nc.dram_tensor("xt", [NBLK, 2, 128, 128], f32, kind="ExternalInput")
    wext = nc.dram_tensor("wext", [IN_FEAT, ZC], f32, kind="ExternalInput")
    gsrc = nc.dram_tensor("gsrc", [128, T], i32, kind="ExternalInput")
    gdst = nc.dram_tensor("gdst", [128, T], i32, kind="ExternalInput")
    dstf = nc.dram_tensor("dstf", [128, T], bf16, kind="ExternalInput")
    iota = nc.dram_tensor("iota", [128, 128], bf16, kind="ExternalInput")
    bias = nc.dram_tensor("bias", [128, 128], bf16, kind="ExternalInput")
    out = nc.dram_tensor("out", [sum(CH_SHARD), OUT_FEAT], f32,
                         kind="ExternalOutput")

    with tile.TileContext(nc) as tc:
        with tc.tile_pool(name="dram", bufs=1, space="DRAM") as dram, \
             tc.tile_pool(name="consts", bufs=1) as cpool, \
             tc.tile_pool(name="ph1", bufs=4) as ph1, \
             tc.tile_pool(name="ph1z", bufs=3) as ph1z, \
             tc.tile_pool(name="ph1p", bufs=2, space="PSUM") as ph1p, \
             tc.tile_pool(name="gat", bufs=4) as gat, \
             tc.tile_pool(name="rhp", bufs=3) as rhp, \
             tc.tile_pool(name="edg", bufs=6) as edg, \
             tc.tile_pool(name="ohp", bufs=3) as ohp, \
             tc.tile_pool(name="ph2p", bufs=6, space="PSUM") as ph2p, \
             tc.tile_pool(name="outp", bufs=6) as outp, \
             tc.tile_pool(name="nrm", bufs=3) as nrm:

            zext = dram.tile([NPAD, ZC], bf16)
            ern = dram.tile([NPAD, 4], bf16)
            s_acc = [dram.tile([CH_ROWS[i], SC], bf16, name=f"s_acc{i}")
                     for i in range(4)]
            s_red = [dram.tile([CH_SHARD[i], SC], bf16, name=f"s_red{i}")
                     for i in range(4)]

            w_sb = cpool.tile([128, 2 * ZC], f32, tag="wext")
            wap = wext[:]
            nc.sync.dma_start(
                w_sb[:], AP(wap.tensor, 0, [[ZC, 128], [128 * ZC, 2], [1, ZC]]))
            iota_sb = cpool.tile([128, 128], bf16, tag="iota")
            nc.sync.dma_start(iota_sb[:], iota[:])
            bias_sb = cpool.tile([128, 128], bf16, tag="bias")
            nc.sync.dma_start(bias_sb[:], bias[:])
            gs_sb = cpool.tile([128, T], i32, tag="gs")
            nc.sync.dma_start(gs_sb[:], gsrc[:])
            gd_sb = cpool.tile([128, T], i32, tag="gd")
            nc.sync.dma_start(gd_sb[:], gdst[:])
            df_sb = cpool.tile([128, T], bf16, tag="df")
            nc.sync.dma_start(df_sb[:], dstf[:])

            xa = xt[:]
            za = zext[:]
            ea = ern[:]

            # ---------------- Phase 1: Zext GEMM ----------------
            for g in range(NBLK // GRP):
                lh = ph1.tile([128, GRP * 2 * 128], f32, tag="lh")
                nc.sync.dma_start(
                    lh[:], AP(xa.tensor, g * GRP * 2 * 128 * 128,
                              [[128, 128], [2 * 128 * 128, GRP],
                               [128 * 128, 2], [1, 128]]))
                zr = ph1z.tile([128, GRP * ZC], bf16, tag="zr")
                for bb in range(GRP):
                    ps = ph1p.tile([128, ZC], f32, space="PSUM", tag="zps")
                    nc.tensor.matmul(out=ps[:],
                                     lhsT=lh[:, bb * 256:bb * 256 + 128],
                                     rhs=w_sb[:, 0:ZC],
                                     start=True, stop=False)
                    nc.tensor.matmul(out=ps[:],
                                     lhsT=lh[:, bb * 256 + 128:bb * 256 + 256],
                                     rhs=w_sb[:, ZC:2 * ZC],
                                     start=False, stop=True)
                    nc.scalar.copy(zr[:, bb * ZC:(bb + 1) * ZC], ps[:])
                zrap = zr[:]
                nc.sync.dma_start(
                    AP(za.tensor, za.offset + g * GRP * 128 * ZC,
                       [[ZC, 128], [128 * ZC, GRP], [1, ZC]]),
                    AP(zrap.tensor, zrap.offset,
                       [zrap.ap[0], [ZC, GRP], [1, ZC]]))
                nc.sync.dma_start(
                    AP(ea.tensor, ea.offset + g * GRP * 128 * 4,
                       [[4, 128], [128 * 4, GRP], [1, 4]]),
                    AP(zrap.tensor, zrap.offset + 132,
                       [zrap.ap[0], [ZC, GRP], [1, 4]]))

            # ---------------- Phase 2: edges ----------------
            ci = 0
            st_hi = CH_ST[0]
            ch_st0 = 0
            for s in range(NST):
                if s >= st_hi:
                    ch_st0 = st_hi
                    ci += 1
                    st_hi += CH_ST[ci]
                G = gat.tile([128, ST * ZC], bf16, tag="G")
                erg = edg.tile([128, ST * 4], bf16, tag="erg")
                for k in range(ST):
                    t = s * ST + k
                    nc.gpsimd.indirect_dma_start(
                        out=G[:, k * ZC:(k + 1) * ZC], out_offset=None,
                        in_=zext[:],
                        in_offset=IndirectOffsetOnAxis(
                            ap=gs_sb[:, t:t + 1], axis=0))
                    nc.gpsimd.indirect_dma_start(
                        out=erg[:, k * 4:(k + 1) * 4], out_offset=None,
                        in_=ern[:],
                        in_offset=IndirectOffsetOnAxis(
                            ap=gd_sb[:, t:t + 1], axis=0))
                gap = G[:]
                el_view = AP(gap.tensor, gap.offset + 128,
                             [gap.ap[0], [ZC, ST], [1, 4]])
                q0 = edg.tile([128, ST * 4], bf16, tag="q0")
                nc.vector.tensor_tensor(q0[:], el_view, erg[:],
                                        op=mybir.AluOpType.add)
                qs = edg.tile([128, ST * 4], bf16, tag="qs")
                nc.vector.tensor_scalar(qs[:], q0[:], LEAKY, None,
                                        op0=mybir.AluOpType.mult)
                q1 = edg.tile([128, ST * 4], bf16, tag="q1")
                nc.vector.tensor_tensor(q1[:], q0[:], qs[:],
                                        op=mybir.AluOpType.max)
                q = edg.tile([128, ST * 4], bf16, tag="q")
                nc.scalar.activation(q[:], q1[:],
                                     mybir.ActivationFunctionType.Exp)
                rhs = rhp.tile([128, ST * SC], bf16, tag="rhs")
                rap = rhs[:]
                qap = q[:]
                nc.vector.tensor_tensor(
                    AP(rap.tensor, rap.offset, [rap.ap[0], [SC, ST], [1, 128]]),
                    AP(gap.tensor, gap.offset, [gap.ap[0], [ZC, ST], [1, 128]]),
                    AP(qap.tensor, qap.offset,
                       [qap.ap[0], [4, ST], [1, 4], [0, HEAD_DIM]]),
                    op=mybir.AluOpType.mult)
                nc.vector.tensor_copy(
                    AP(rap.tensor, rap.offset + 128,
                       [rap.ap[0], [SC, ST], [1, 4]]),
                    qap)

                oh = ohp.tile([128, ST * 128], bf16, tag="oh")
                iap = iota_sb[:]
                dap = df_sb[:]
                nc.vector.tensor_tensor(
                    oh[:],
                    AP(iap.tensor, iap.offset, [iap.ap[0], [0, ST], [1, 128]]),
                    AP(dap.tensor, dap.offset + s * ST,
                       [dap.ap[0], [1, ST], [0, 128]]),
                    op=mybir.AluOpType.is_equal)

                pt = [ph2p.tile([128, 3 * SC], f32, space="PSUM", tag="sps",
                                name=f"pt{j}")
                      for j in range(3)]
                for k in range(ST):
                    blk = k // TPB
                    dst = pt[blk // 3][:, (blk % 3) * SC:(blk % 3 + 1) * SC]
                    nc.tensor.matmul(
                        out=dst, lhsT=oh[:, k * 128:(k + 1) * 128],
                        rhs=rhs[:, k * SC:(k + 1) * SC],
                        start=(k % TPB == 0), stop=(k % TPB == TPB - 1))
                sow = outp.tile([128, BPS * SC], bf16, tag="sow")
                for j in range(3):
                    w = (3 * SC) if j < 2 else (2 * SC)
                    nc.scalar.copy(sow[:, j * 3 * SC:j * 3 * SC + w],
                                   pt[j][:, :w])
                sa = s_acc[ci][:]
                soap = sow[:]
                nc.sync.dma_start(
                    AP(sa.tensor, sa.offset + (s - ch_st0) * BPS * 128 * SC,
                       [[SC, 128], [128 * SC, BPS], [1, SC]]),
                    AP(soap.tensor, soap.offset,
                       [soap.ap[0], [SC, BPS], [1, SC]]))
                if s == st_hi - 1:
                    nc.gpsimd.collective_compute(
                        "ReduceScatter", mybir.AluOpType.add,
                        replica_groups=[list(range(NCORES))],
                        ins=[s_acc[ci][:].opt()], outs=[s_red[ci][:].opt()])

            # ---------------- Phase 3: normalize ----------------
            oa = out[:]
            for i in range(4):
                sr = s_red[i][:]
                nblk_i = CH_SHARD[i] // 128
                g0 = 0
                while g0 < nblk_i:
                    nb = min(7, nblk_i - g0)
                    stt = nrm.tile([128, 7 * SC], bf16, tag="stt")
                    nc.sync.dma_start(
                        stt[:, :nb * SC],
                        AP(sr.tensor, sr.offset + g0 * 128 * SC,
                           [[SC, 128], [128 * SC, nb], [1, SC]]))
                    sta = stt[:]
                    dg = nrm.tile([128, 7 * 4], bf16, tag="dg")
                    nc.vector.tensor_scalar(
                        dg[:, :nb * 4],
                        AP(sta.tensor, sta.offset + 128,
                           [sta.ap[0], [SC, nb], [1, 4]]),
                        1e-30, None, op0=mybir.AluOpType.max)
                    rc = nrm.tile([128, 7 * 4], bf16, tag="rc")
                    with nc.allow_low_precision(reason="bf16 recip ok at tol"):
                        nc.vector.reciprocal(rc[:, :nb * 4], dg[:, :nb * 4])
                    rca = rc[:]
                    ot = nrm.tile([128, 7 * 128], bf16, tag="ot")
                    nc.vector.tensor_tensor(
                        ot[:, :nb * 128],
                        AP(sta.tensor, sta.offset,
                           [sta.ap[0], [SC, nb], [1, 128]]),
                        AP(rca.tensor, rca.offset,
                           [rca.ap[0], [4, nb], [1, 4], [0, HEAD_DIM]]),
                        op=mybir.AluOpType.mult)
                    ot2 = nrm.tile([128, 7 * 128], f32, tag="ot2")
                    bap = bias_sb[:]
                    nc.vector.tensor_tensor(
                        ot2[:, :nb * 128],
                        ot[:, :nb * 128],
                        AP(bap.tensor, bap.offset,
                           [bap.ap[0], [0, nb], [1, 128]]),
                        op=mybir.AluOpType.add)
                    o2a = ot2[:]
                    nc.sync.dma_start(
                        AP(oa.tensor,
                           (CH_OUT0[i] + g0 * 128) * OUT_FEAT,
                           [[OUT_FEAT, 128], [128 * OUT_FEAT, nb],
                            [1, OUT_FEAT]]),
                        AP(o2a.tensor, o2a.offset,
                           [o2a.ap[0], [OUT_FEAT, nb], [1, OUT_FEAT]]))
                    g0 += nb

    nc.compile()
    return nc


# ------------------------------------------------------------------- entry
def kernel(inputs, conv_weights, attn_l, attn_r, h_bias, row_idx, col_idx,
           _trace=False, _tmpdir=None):
    xt, per_core, consts = _prep(inputs, conv_weights, attn_l, attn_r,
                                 h_bias, row_idx, col_idx)
    if 'nc' not in _CACHE:
        _CACHE['nc'] = _build()
    nc = _CACHE['nc']

    in_maps = []
    for r in range(NCORES):
        pc = per_core[r]
        in_maps.append(dict(
            xt=xt, wext=pc['wext'],
            gsrc=pc['gsrc'], gdst=pc['gdst'], dstf=pc['dstf'],
            iota=consts['iota'], bias=consts['bias'],
        ))

    from concourse import bass_utils
    res = bass_utils.run_bass_kernel_spmd(
        nc, in_maps, core_ids=list(range(NCORES)),
        trace=_trace, **({'tmpdir': _tmpdir} if _tmpdir else {}))
    full = np.empty((NPAD, OUT_FEAT), np.float32)
    ch_row0 = np.cumsum([0] + CH_ROWS[:-1])
    for c in range(NCORES):
        oc = np.asarray(res.results[c]['out'], np.float32)
        for i in range(4):
            gl0 = ch_row0[i] + c * CH_SHARD[i]
            full[gl0:gl0 + CH_SHARD[i]] = (
                oc[CH_OUT0[i]:CH_OUT0[i] + CH_SHARD[i]])
    kernel.last_result = res
    return full[:N_NODES]
